# revision 21
# baseline (speedup 1.0000x reference)
"""Bass/Trainium2 SPMD kernel for GQA causal attention with RoPE.

Sharding (8 cores): core c = 4*b + j (b = batch, j = 0..3 shard in batch).
  - Q / attention / o_proj: token-sharded; core j owns q-token 128-blocks
    {j, 7-j, j+8, 15-j} (balanced causal work, uniform SPMD program with
    fixed per-slot key extents [512, 1024, 1536, 2048] and data-driven masks).
  - K: head-sharded (core j computes kv-heads 2j, 2j+1 for all T, with RoPE),
    V: token-sharded (core j computes tokens [512j, 512j+512), all dims).
    One fused AllGather per 4-core group assembles full K^T and V.
  - Attention in s^T = (kpos, q) layout: scores = k^T.T @ q^T, exp on ACT
    (scale folds 1/sqrt(hd)), AV with ones-augmented V gives softmax sums,
    division applied after AV (commutes with the linear AV/o_proj steps).
All matmuls run in fp32r (full-rate fp32 on TRN2 at free-dim >= 256).
"""
import numpy as np

import concourse.bass as bass
import concourse.tile as tile
from concourse import bacc, mybir
from concourse.bass_utils import run_bass_kernel_spmd

B, T, D = 2, 2048, 2048
H, KV, HD = 32, 8, 64
P = 128
NB = T // P          # 16 token blocks of 128
OWN = 4 * P          # 512 owned q tokens per core
f32 = mybir.dt.float32
f32r = mybir.dt.float32r
Exp = mybir.ActivationFunctionType.Exp

KVE = 2 * HD         # 128 kv dims computed per core (2 kv heads)
AG_K = P * T         # 262144 elems of kT shard
AG_V = 512 * 512     # 262144 elems of v shard
AG_N = AG_K + AG_V


def _qblocks(j):
    return [j, 7 - j, j + 8, 15 - j]


def _build():
    nc = bacc.Bacc("TRN2", target_bir_lowering=False, debug=False, num_devices=8)

    xt = nc.dram_tensor("xt", [D, T], f32, kind="ExternalInput").ap()
    xtq = nc.dram_tensor("xtq", [D, OWN], f32, kind="ExternalInput").ap()
    xtv = nc.dram_tensor("xtv", [D, 512], f32, kind="ExternalInput").ap()
    wqt = nc.dram_tensor("wqt", [D, H * HD], f32, kind="ExternalInput").ap()
    wkt = nc.dram_tensor("wkt", [D, KVE], f32, kind="ExternalInput").ap()
    wvt = nc.dram_tensor("wvt", [D, KV * HD], f32, kind="ExternalInput").ap()
    wot = nc.dram_tensor("wot", [H * HD, D], f32, kind="ExternalInput").ap()
    cost = nc.dram_tensor("cost", [HD, T], f32, kind="ExternalInput").ap()
    sint = nc.dram_tensor("sint", [HD, T], f32, kind="ExternalInput").ap()
    costq = nc.dram_tensor("costq", [HD, OWN], f32, kind="ExternalInput").ap()
    sintq = nc.dram_tensor("sintq", [HD, OWN], f32, kind="ExternalInput").ap()
    maskb = nc.dram_tensor("maskb", [NB, P, P], f32, kind="ExternalInput").ap()
    onesr = nc.dram_tensor("onesr", [1, HD], f32, kind="ExternalInput").ap()
    onesc = nc.dram_tensor("onesc", [P, NB], f32, kind="ExternalInput").ap()
    y = nc.dram_tensor("y", [OWN, D], f32, kind="ExternalOutput").ap()

    def rope_pair(dst, src, cosr, sinr, po, t0, t1, tmp_pool, n):
        """dst[po:po+64, t0:t1] = rope(src rows [po,po+64)); cosr/sinr (64, n)."""
        a1 = src[po:po + 32, :]
        a2 = src[po + 32:po + 64, :]
        u = tmp_pool.tile([32, 512], f32, tag="ropeu", bufs=4, name="u")
        v = tmp_pool.tile([32, 512], f32, tag="ropev", bufs=4, name="v")
        nc.vector.tensor_mul(u[:, :n], a1, cosr[0:32, :])
        nc.vector.tensor_mul(v[:, :n], a2, sinr[0:32, :])
        nc.vector.tensor_sub(dst[po:po + 32, t0:t1], u[:, :n], v[:, :n])
        u2 = tmp_pool.tile([32, 512], f32, tag="ropeu", bufs=4, name="u2")
        v2 = tmp_pool.tile([32, 512], f32, tag="ropev", bufs=4, name="v2")
        nc.vector.tensor_mul(u2[:, :n], a2, cosr[32:64, :])
        nc.vector.tensor_mul(v2[:, :n], a1, sinr[32:64, :])
        nc.vector.tensor_add(dst[po + 32:po + 64, t0:t1], u2[:, :n], v2[:, :n])

    with tile.TileContext(nc) as tc:
        _dpool_cm = tc.tile_pool(name="dram", bufs=1, space="DRAM")
        dpool = _dpool_cm.__enter__()
        _pers_cm = tc.tile_pool(name="pers", bufs=1)
        pers = _pers_cm.__enter__()

        agin = dpool.tile([AG_N], f32, tag="agin", name="agin")
        agout = dpool.tile([4, AG_N], f32, tag="agout", name="agout")
        obuf = dpool.tile([H * HD, OWN], f32, tag="obuf", name="obuf")

        # persistent across stages: q cos/sin, qTr, ones
        cosq_t = pers.tile([HD, OWN], f32, tag="cosq_t", name="cosq_t")
        sinq_t = pers.tile([HD, OWN], f32, tag="sinq_t", name="sinq_t")
        ones_t = pers.tile([1, HD], f32r, tag="ones_t", name="ones_t")
        nc.sync.dma_start(cosq_t[:], costq[:])
        nc.sync.dma_start(sinq_t[:], sintq[:])
        nc.sync.dma_start(ones_t[:], onesr[:].bitcast(f32r))
        qtr = [pers.tile([P, OWN], f32r, tag=f"qtr{i}", name=f"qtr{i}")
               for i in range(16)]

        # ================= stages A-D: projections + AllGather launch ======
        with tc.tile_pool(name="consA", bufs=1) as cA, \
             tc.tile_pool(name="wk", bufs=1) as wkp, \
             tc.tile_pool(name="wst", bufs=3) as wst, \
             tc.tile_pool(name="xs", bufs=3) as xsp, \
             tc.tile_pool(name="tmp", bufs=4) as tmpp, \
             tc.tile_pool(name="cpy", bufs=3) as cpyp, \
             tc.tile_pool(name="pproj", bufs=1, space="PSUM") as pproj:

            cosf_t = cA.tile([HD, T], f32, tag="cosf_t", name="cosf_t")
            sinf_t = cA.tile([HD, T], f32, tag="sinf_t", name="sinf_t")
            ktr_sh = cA.tile([P, T], f32, tag="ktr_sh", name="ktr_sh")
            nc.sync.dma_start(cosf_t[:], cost[:])
            nc.sync.dma_start(sinf_t[:], sint[:])

            # ---- stage A: K projection (head shard, full T) + rope ----
            wk_sb = []
            for cb in range(16):
                wt = wkp.tile([P, KVE], f32r, tag=f"wk{cb}", name=f"wk{cb}")
                nc.sync.dma_start(wt[:], wkt[P * cb:P * cb + P, :].bitcast(f32r))
                wk_sb.append(wt)
            for tb in range(4):
                ps = pproj.tile([P, 512], f32, tag="psk", bufs=2, name="psk")
                for cb in range(16):
                    xt_t = xsp.tile([P, 512], f32r, tag="xtk", name="xtk")
                    nc.sync.dma_start(
                        xt_t[:], xt[P * cb:P * cb + P,
                                    512 * tb:512 * tb + 512].bitcast(f32r))
                    nc.tensor.matmul(ps[:], lhsT=wk_sb[cb], rhs=xt_t[:],
                                     start=(cb == 0), stop=(cb == 15))
                cs = cosf_t[:, 512 * tb:512 * tb + 512]
                sn = sinf_t[:, 512 * tb:512 * tb + 512]
                for po in (0, 64):
                    rope_pair(ktr_sh, ps, cs, sn, po,
                              512 * tb, 512 * tb + 512, tmpp, 512)
            nc.sync.dma_start(
                agin[0:AG_K].rearrange("(p t) -> p t", t=T), ktr_sh[:])

            # ---- stage B: V projection (token shard), cb-outer ----
            psv = [pproj.tile([P, 512], f32, tag=f"acc{i}", bufs=1,
                              name=f"psv{i}") for i in range(4)]
            for cb in range(16):
                wv_t = wst.tile([P, KV * HD], f32r, tag="wv", name="wv")
                nc.sync.dma_start(wv_t[:], wvt[P * cb:P * cb + P, :].bitcast(f32r))
                for vb in range(4):
                    xv_t = xsp.tile([P, P], f32r, tag="xtv", name="xtv")
                    nc.sync.dma_start(
                        xv_t[:], xtv[P * cb:P * cb + P,
                                     P * vb:P * vb + P].bitcast(f32r))
                    nc.tensor.matmul(psv[vb][:], lhsT=xv_t[:], rhs=wv_t[:],
                                     start=(cb == 0), stop=(cb == 15))
            for vb in range(4):
                vs = cpyp.tile([P, 512], f32, tag="vsh", name="vsh")
                nc.scalar.copy(vs[:], psv[vb][:])
                nc.sync.dma_start(
                    agin[AG_K + vb * P * 512:
                         AG_K + (vb + 1) * P * 512].rearrange(
                             "(p t) -> p t", t=512), vs[:])

            # ---- stage C: fused AllGather of (kT shard | v shard) ----
            nc.gpsimd.collective_compute(
                "AllGather",
                mybir.AluOpType.bypass,
                replica_groups=[[0, 1, 2, 3], [4, 5, 6, 7]],
                ins=[agin.opt()],
                outs=[agout.opt()],
            )

            # ---- stage D: Q projection (owned tokens) + rope ----
            xtq_sb = []
            for cb in range(16):
                wt = wkp.tile([P, OWN], f32r, tag=f"xq{cb}", name=f"xq{cb}")
                nc.sync.dma_start(wt[:], xtq[P * cb:P * cb + P, :].bitcast(f32r))
                xtq_sb.append(wt)
            for qg in range(4):
                psq = [pproj.tile([P, 512], f32, tag=f"acc{i}", bufs=1,
                                  name=f"psq{i}") for i in range(4)]
                for cb in range(16):
                    wq_t = wst.tile([P, 512], f32r, tag="wq", name="wq")
                    nc.sync.dma_start(
                        wq_t[:], wqt[P * cb:P * cb + P,
                                     512 * qg:512 * qg + 512].bitcast(f32r))
                    for qi in range(4):
                        nc.tensor.matmul(
                            psq[qi][:], lhsT=wq_t[:, P * qi:P * qi + P],
                            rhs=xtq_sb[cb], start=(cb == 0), stop=(cb == 15))
                for qi in range(4):
                    for po in (0, 64):
                        rope_pair(qtr[4 * qg + qi], psq[qi], cosq_t, sinq_t,
                                  po, 0, OWN, tmpp, OWN)

        # ================= stages E-F: unpack AG + attention ================
        with tc.tile_pool(name="persF", bufs=1) as pF, \
             tc.tile_pool(name="psS", bufs=3, space="PSUM") as psS, \
             tc.tile_pool(name="psO", bufs=2, space="PSUM") as psO, \
             tc.tile_pool(name="psB", bufs=2, space="PSUM") as psB, \
             tc.tile_pool(name="ptp", bufs=3) as ptp, \
             tc.tile_pool(name="nrm", bufs=2) as nrm:

            mask_t = pF.tile([P, NB * P], f32, tag="mask_t", name="mask_t")
            for kb in range(NB):
                nc.sync.dma_start(mask_t[:, P * kb:P * kb + P], maskb[kb])
            ktr = [pF.tile([P, T], f32r, tag=f"ktr{g}", name=f"ktr{g}")
                   for g in range(4)]
            vaug = [pF.tile([P, NB * (HD + 1)], f32r, tag=f"vaug{kh}",
                            name=f"vaug{kh}") for kh in range(KV)]
            for g in range(4):
                nc.sync.dma_start(
                    ktr[g][:],
                    agout[g, 0:AG_K].rearrange("(p t) -> p t", t=T).bitcast(f32r))
            for kh in range(KV):
                for kb in range(NB):
                    g, lr = kb // 4, (kb % 4) * P
                    vsrc = agout[g, AG_K:AG_N].rearrange("(t v) -> t v", v=512)
                    nc.sync.dma_start(
                        vaug[kh][:, (HD + 1) * kb:(HD + 1) * kb + HD],
                        vsrc[lr:lr + P, HD * kh:HD * kh + HD].bitcast(f32r))
                ocol = vaug[kh].rearrange("p (k c) -> p k c", c=HD + 1)[:, :, HD]
                nc.sync.dma_start(ocol, onesc[:].bitcast(f32r))

            for h in range(H):
                kh = h // 4
                kt, kpo = ktr[kh // 2], HD * (kh % 2)
                # q heads are host-permuted: tile u holds head 8*(u//4)+u%4
                # at rows 0:64 (kv even) and that head +4 at rows 64:128.
                u = (kh // 2) * 4 + h % 4
                qt, qpo = qtr[u], HD * (kh % 2)
                oslot = 2 * u + (kh % 2)
                oaug = psO.tile([P, OWN], f32, tag="oaug", name="oaug")
                for kb in range(NB):
                    qs = P * (kb // 4)
                    n = OWN - qs
                    st = psS.tile([P, OWN], f32, tag="st", name="st")
                    nc.tensor.matmul(
                        st[:, 0:n],
                        lhsT=kt[kpo:kpo + HD, P * kb:P * kb + P],
                        rhs=qt[qpo:qpo + HD, qs:OWN],
                        start=True, stop=True)
                    nc.vector.tensor_add(st[:, 0:P], st[:, 0:P],
                                         mask_t[:, P * kb:P * kb + P])
                    pt = ptp.tile([P, OWN], f32r, tag="pt", name="pt")
                    nc.scalar.activation(pt[:, 0:n], st[:, 0:n], Exp, scale=0.125)
                    nc.tensor.matmul(
                        oaug[0:HD + 1, qs:OWN],
                        lhsT=vaug[kh][:, (HD + 1) * kb:(HD + 1) * (kb + 1)],
                        rhs=pt[:, 0:n],
                        start=(kb == 0), stop=(kb == 15))
                rec = nrm.tile([1, OWN], f32, tag="rec", name="rec")
                nc.vector.reciprocal(rec[:], oaug[HD:HD + 1, :])
                recr = nrm.tile([1, OWN], f32r, tag="recr", name="recr")
                nc.scalar.copy(recr[:], rec[:])
                pb = psB.tile([HD, OWN], f32, tag="pb", name="pb")
                nc.tensor.matmul(pb[:], lhsT=ones_t[:], rhs=recr[:],
                                 start=True, stop=True)
                pbs = nrm.tile([HD, OWN], f32, tag="pbs", bufs=2, name="pbs")
                nc.vector.tensor_copy(pbs[:], pb[:])
                otmp = nrm.tile([HD, OWN], f32, tag="otmp", bufs=3, name="otmp")
                nc.vector.tensor_mul(otmp[:], oaug[0:HD, :], pbs[:])
                nc.sync.dma_start(obuf[HD * oslot:HD * oslot + HD, :], otmp[:])

        # ================= stage G: o_proj ================================
        with tc.tile_pool(name="wos", bufs=3) as wos, \
             tc.tile_pool(name="otrg", bufs=1) as otrp, \
             tc.tile_pool(name="psG", bufs=1, space="PSUM") as psG, \
             tc.tile_pool(name="yc", bufs=3) as ycp:
            otr = []
            for ab in range(16):
                wt = otrp.tile([P, OWN], f32r, tag=f"otr{ab}", name=f"otr{ab}")
                nc.sync.dma_start(wt[:],
                                  obuf[P * ab:P * ab + P, :].bitcast(f32r))
                otr.append(wt)
            for eb in range(4):
                psg = [psG.tile([P, 512], f32, tag=f"psg{i}", bufs=1,
                                name=f"psg{i}") for i in range(4)]
                for ab in range(16):
                    wo_t = wos.tile([P, 512], f32r, tag="wo", name="wo")
                    nc.sync.dma_start(
                        wo_t[:], wot[P * ab:P * ab + P,
                                     512 * eb:512 * eb + 512].bitcast(f32r))
                    for tb in range(4):
                        nc.tensor.matmul(
                            psg[tb][:], lhsT=otr[ab][:, P * tb:P * tb + P],
                            rhs=wo_t[:], start=(ab == 0), stop=(ab == 15))
                for tb in range(4):
                    yt = ycp.tile([P, 512], f32, tag="yt", name="yt")
                    nc.scalar.copy(yt[:], psg[tb][:])
                    nc.sync.dma_start(
                        y[P * tb:P * tb + P, 512 * eb:512 * eb + 512], yt[:])

        _pers_cm.__exit__(None, None, None)
        _dpool_cm.__exit__(None, None, None)

    nc.compile()
    return nc


_NC = None


def _get_nc():
    global _NC
    if _NC is None:
        _NC = _build()
    return _NC


def _head_perm():
    """Pair each even-kv head with its odd-kv partner (+4) in one 128-dim
    block, so q partition parity matches the kv head parity in kT tiles."""
    order = []
    for u in range(16):
        a = 8 * (u // 4) + u % 4
        for h in (a, a + 4):
            order.extend(range(HD * h, HD * h + HD))
    return np.asarray(order)


def _in_maps(x, cos, sin, Wq, Wk, Wv, Wo):
    xT = np.ascontiguousarray(np.transpose(np.asarray(x, np.float32), (0, 2, 1)))
    perm = _head_perm()
    WqT = np.ascontiguousarray(np.asarray(Wq, np.float32).T[:, perm])
    WkT = np.ascontiguousarray(np.asarray(Wk, np.float32).T)
    WvT = np.ascontiguousarray(np.asarray(Wv, np.float32).T)
    WoT = np.ascontiguousarray(np.asarray(Wo, np.float32).T[perm, :])
    cosT = np.ascontiguousarray(np.asarray(cos, np.float32).T)
    sinT = np.ascontiguousarray(np.asarray(sin, np.float32).T)
    ones = np.ones((1, HD), np.float32)
    maps = []
    for c in range(8):
        b, j = c // 4, c % 4
        qb = _qblocks(j)
        cols = np.concatenate([np.arange(P * g, P * g + P) for g in qb])
        mask = np.empty((NB, P, P), np.float32)
        ki = np.arange(P)[:, None]
        qi = np.arange(P)[None, :]
        for kb in range(NB):
            qg = qb[kb // 4]
            mask[kb] = np.where(P * kb + ki <= P * qg + qi, 0.0, -1e9)
        maps.append({
            "xt": xT[b],
            "xtq": np.ascontiguousarray(xT[b][:, cols]),
            "xtv": np.ascontiguousarray(xT[b][:, 512 * j:512 * j + 512]),
            "wqt": WqT,
            "wkt": np.ascontiguousarray(WkT[:, KVE * j:KVE * j + KVE]),
            "wvt": WvT,
            "wot": WoT,
            "cost": cosT, "sint": sinT,
            "costq": np.ascontiguousarray(cosT[:, cols]),
            "sintq": np.ascontiguousarray(sinT[:, cols]),
            "maskb": mask,
            "onesr": ones,
            "onesc": np.ones((P, NB), np.float32),
        })
    return maps


def kernel(x, cos, sin, Wq, Wk, Wv, Wo):
    nc = _get_nc()
    maps = _in_maps(x, cos, sin, Wq, Wk, Wv, Wo)
    res = run_bass_kernel_spmd(nc, maps, list(range(8)))
    out = np.empty((B, T, D), np.float32)
    for c in range(8):
        b, j = c // 4, c % 4
        yc = res.results[c]["y"]
        for s, qg in enumerate(_qblocks(j)):
            out[b, P * qg:P * qg + P, :] = yc[P * s:P * s + P, :]
    return out
